# revision 86
# baseline (speedup 1.0000x reference)
"""Trainium2 Bass kernel for multi-head self-attention with RoPE.

Problem shapes (hardcoded): x [2, 2048, 1024], 16 heads x 64 dim, fp32.
Sharding: 2-D tensor parallel -- core c owns batch c//4 and the 4 heads
[4*(c%4), 4*(c%4)+4).  Each core computes q/k/v projections for its head
quad, RoPE, dense attention over the full 2048x2048 score matrix, and its
partial output projection in bf16; the host sums the 4 fp16 partials per
batch and adds bo (with bv folded in exactly via bo' = bo + bv @ Wo).

Device-side layout notes:
 - Matmul contractions need the contracted dim on SBUF partitions, so the
   host passes x pre-transposed per batch (xt [D, T], bf16).
 - Q^T/K^T are produced in [head_dim, T] bf16 layout; RoPE uses an
   interleaved pair layout (W columns permuted on host) so the rotation
   partner of partition p is p^1 (stream_shuffle within-32 permutation).
   The shuffle + cos-mul + final add run on DVE, the sin mul on GpSimd.
   Nonzero q/k biases are folded into a host-precomputed additive table
   (b*cos + swap(b)*sin); the kernel is built without the extra add when
   the biases are all zero (the build is keyed on that).
 - V is computed directly in [token, dim] layout (x^T chunks stationary)
   so no PE transposes are needed; a strided DVE copy drops the four
   heads' 64-wide slices into a [128, 16*260] arena with interleaved
   ones columns for the softmax denominators.
 - Scores are computed transposed (S^T = K Q^T, [k, q]) with the two heads
   of a pair in different PE row groups; one [128,1024] exp on the scalar
   engine covers both.  PV accumulates ctx^T [65, q] (ones row = denom).
 - Normalization: reciprocal on DVE, partition move via SBUF-SBUF DMA,
   broadcast on GpSimd, multiply on DVE; head 1 of each pair reaches
   partitions 64:128 of the stacked ctx via an SBUF-SBUF DMA.
"""

import os
import numpy as np

import concourse.bass as bass
import concourse.tile as tile
from concourse import mybir
from concourse.bass_utils import run_bass_kernel_spmd

N_CORES = 8
B, T, D = 2, 2048, 1024
H, HD = 16, 64            # total heads, head dim
HL = 4                    # heads per core
DL = HL * HD              # local head dims (256)
CC = D // 128             # contraction chunks (8)
NT = T // 512             # 512-wide t-chunks (4)
NKT = T // 128            # 128-row k-tiles (16)
VW = HD + 1               # per-head vh block width (65)
F32 = mybir.dt.float32
F16 = mybir.dt.float16
BF = mybir.dt.bfloat16
FR = mybir.dt.float32r
I16 = mybir.dt.int16

# within-32 adjacent-pair swap for RoPE (partition p <-> p^1)
SWAP_MASK = [i ^ 1 for i in range(32)]

# k-tiles whose exp runs on DVE via the bf16 Schraudolph bit trick
# (bits = score * 128*log2(e)/8 + (127*128 - C)); C is calibrated so the
# approximation is mean-zero against the exact exp, which makes the
# constant part cancel in the softmax normalization.
# per-hp: chosen to dodge each block's DVE bursts (norm steps at kt 1-4,
# deferred-Q rope at ~4-8 on hp1, out-proj casts at 8-11 on hp0); block 0
# is excluded (its DVE does the V copies)
DVE_EXP_KT = {0: (5, 13), 1: (2, 12)}
SCHRAU_S1 = 128.0 / np.log(2.0) / 8.0
SCHRAU_S2 = 16256.0 - 7.363

_CACHE = {}
LAST_RESULT = None


def _build_nc(with_qk_bias=False, dbg_names=()):
    from concourse import bacc
    nc = bacc.Bacc("TRN2", target_bir_lowering=False, debug=False,
                   num_devices=N_CORES)
    xt = nc.dram_tensor("xt", [D, T], BF, kind="ExternalInput").ap()
    wq = nc.dram_tensor("wq", [D, DL], BF, kind="ExternalInput").ap()
    wk = nc.dram_tensor("wk", [D, DL], BF, kind="ExternalInput").ap()
    wv = nc.dram_tensor("wv", [D, DL], BF, kind="ExternalInput").ap()
    wo2 = nc.dram_tensor("wo2", [DL, D], BF, kind="ExternalInput").ap()
    cosb = nc.dram_tensor("cosb", [128, T], BF, kind="ExternalInput").ap()
    sinb = nc.dram_tensor("sinb", [128, T], BF, kind="ExternalInput").ap()
    # additive rope bias tables per m-tile (b*cos + swap(b)*sin), only
    # read when with_qk_bias: [mt, q/k] -> [128, T]
    bias_t = None
    if with_qk_bias:
        bias_t = nc.dram_tensor("bias_t", [128, 4 * T], F32,
                                kind="ExternalInput").ap()
    out = nc.dram_tensor("out", [T, D], F16, kind="ExternalOutput").ap()

    dbg = {}
    dbg_shapes = {
        "dbg_qrot": [128, T], "dbg_krot": [128, T], "dbg_vh": [128, NKT * 260],
        "dbg_pt": [128, 1024], "dbg_cx": [VW, 512],
    }
    for n in dbg_names:
        dbg[n] = nc.dram_tensor(n, dbg_shapes[n], F32,
                                kind="ExternalOutput").ap()

    with tile.TileContext(nc) as tc:
        _body(tc, xt, wq, wk, wv, wo2, cosb, sinb, bias_t, out, dbg)

    nc.compile()
    return nc


def _body(tc, xt, wq, wk, wv, wo2, cosb, sinb, bias_t, out, dbg={}):
    nc = tc.nc
    AF = mybir.ActivationFunctionType
    OP = mybir.AluOpType
    from contextlib import ExitStack
    with ExitStack() as ctx:
        consts = ctx.enter_context(tc.tile_pool(name="consts", bufs=1))
        persist = ctx.enter_context(tc.tile_pool(name="persist", bufs=1))
        sm_pool = ctx.enter_context(tc.tile_pool(name="sm", bufs=3))
        p_pool = ctx.enter_context(tc.tile_pool(name="pp", bufs=4))
        nrm_pool = ctx.enter_context(tc.tile_pool(name="nrm", bufs=3))
        stk_pool = ctx.enter_context(tc.tile_pool(name="stk", bufs=2))
        out_pool = ctx.enter_context(tc.tile_pool(name="outp", bufs=3))
        # PSUM: ps_sp tag "sp" 3x2 banks + ps_cx 1x2 banks = 8 banks.
        # pp/pv/op all use "sp"-tagged [128,1024] tiles.  cx is single-
        # buffered but freed by one bulk copy right after its last PV.
        ps_sp = ctx.enter_context(tc.tile_pool(name="pse", bufs=3,
                                               space="PSUM"))
        ps_cx = ctx.enter_context(tc.tile_pool(name="psc", bufs=1,
                                               space="PSUM"))

        # ---- constants (DMA order matters: the sync queue is FIFO, and
        # the K projection + first attention block gate the whole span) ----
        wq_sb = consts.tile([128, CC * DL], BF)
        wk_sb = consts.tile([128, CC * DL], BF)
        wv_sb = consts.tile([128, CC * DL], BF)
        wo_sb = consts.tile([128, 2 * D], BF)
        wo_lo = consts.tile([HD, 2 * D], BF)   # rows 64:128, at part. 0:64
        cos_sb = consts.tile([128, T], BF)
        sin_sb = consts.tile([128, T], BF)
        bias_sb = (consts.tile([128, 4 * T], F32)
                   if bias_t is not None else None)

        ones_bf = consts.tile([128, 512], BF)
        nc.gpsimd.memset(ones_bf[:, :], 1.0)

        # persistent per-batch tensors
        qrot = [persist.tile([128, T], BF, tag=f"qrot{mt}",
                             name=f"qrot{mt}") for mt in range(2)]
        krot = [persist.tile([128, T], BF, tag=f"krot{mt}",
                             name=f"krot{mt}") for mt in range(2)]
        vh = persist.tile([128, NKT * HL * VW], BF, tag="vh")
        # ones columns (col 64 of each 65-wide head block)
        nc.gpsimd.memset(
            vh[:, :].rearrange("p (kt h j) -> p kt h j", h=HL, j=VW)
            [:, :, :, HD:HD + 1], 1.0)

        # x^T stays resident in SBUF (16 KiB/partition in bf16), loaded in
        # four t-chunk slices so the K projection can start after the first
        xt_sb = [persist.tile([128, CC * 512], BF, tag=f"xt{tcn}",
                              name=f"xt{tcn}") for tcn in range(NT)]
        xt_src = xt.rearrange("(cc p) t -> p cc t", p=128)

        def xt_load(tcn, split=False):
            # split=True issues odd chunks on the gpsimd DMA queue so the
            # per-instruction issue cost (~0.6us each) parallelizes
            for ci in range(CC):
                eng = nc.gpsimd if (split and ci % 2) else nc.sync
                eng.dma_start(
                    xt_sb[tcn][:, ci * 512:(ci + 1) * 512],
                    xt_src[:, ci, tcn * 512:(tcn + 1) * 512])

        def w_load(w_sb, w, mt, eng=None):
            # one 128-wide m-tile of a projection weight (all CC chunks)
            (eng or nc.sync).dma_start(
                w_sb[:, :].rearrange("p (cc m) -> p cc m", cc=CC)
                [:, :, mt * 128:(mt + 1) * 128],
                w.rearrange("(cc p) m -> p cc m", p=128)
                [:, :, mt * 128:(mt + 1) * 128])

        w_load(wk_sb, wk, 0)
        w_load(wq_sb, wq, 0, eng=nc.gpsimd)
        xt_load(0, split=True)
        nc.sync.dma_start(cos_sb[:, 0:1024], cosb[:, 0:1024])
        nc.sync.dma_start(sin_sb[:, 0:1024], sinb[:, 0:1024])
        nc.sync.dma_start(
            wv_sb[:, :].rearrange("p (cc m) -> p cc m", cc=CC),
            wv.rearrange("(cc p) m -> p cc m", p=128))
        nc.sync.dma_start(cos_sb[:, 1024:T], cosb[:, 1024:T])
        nc.sync.dma_start(sin_sb[:, 1024:T], sinb[:, 1024:T])
        if bias_sb is not None:
            nc.sync.dma_start(bias_sb[:, :], bias_t)
        xt_load(1)
        w_load(wk_sb, wk, 1)
        w_load(wq_sb, wq, 1)

        def qk_proj(name, mt, tcn, dve_rope=False):
            """project one 128-dim m-tile for one 512-col t-chunk + RoPE."""
            w_sb = wq_sb if name == "q" else wk_sb
            dsts = qrot if name == "q" else krot
            ts = slice(tcn * 512, (tcn + 1) * 512)
            pp = ps_sp.tile([128, 1024], F32, tag="sp",
                            name=f"pp_{name}{mt}_{tcn}")
            for ci in range(CC):
                nc.tensor.matmul(
                    pp[:, 0:512],
                    w_sb[:, ci * DL + mt * 128:ci * DL + mt * 128 + 128],
                    xt_sb[tcn][:, ci * 512:(ci + 1) * 512],
                    start=(ci == 0), stop=(ci == CC - 1))
            # rot = pp * cos + swap(pp) * sin [+ bias table]
            # DVE: shuffle + cos-mul (PSUM reads) + add; GpSimd: sin-mul
            shuf = sm_pool.tile([128, 512], F32, tag="shuf")
            nc.vector.stream_shuffle(shuf[:, :], pp[:, 0:512], SWAP_MASK)
            ca = sm_pool.tile([128, 512], BF, tag="ca")
            nc.vector.tensor_mul(ca[:, :], pp[:, 0:512], cos_sb[:, ts])
            sa = sm_pool.tile([128, 512], BF, tag="sa")
            sa_eng = nc.vector if dve_rope else nc.gpsimd
            sa_eng.tensor_mul(sa[:, :], shuf[:, :], sin_sb[:, ts])
            if bias_sb is None:
                nc.vector.tensor_add(dsts[mt][:, ts], ca[:, :], sa[:, :])
            else:
                bci = (mt * 2 + (0 if name == "q" else 1)) * T
                rsum = sm_pool.tile([128, 512], BF, tag="rsum")
                nc.vector.tensor_add(rsum[:, :], ca[:, :], sa[:, :])
                nc.vector.tensor_add(
                    dsts[mt][:, ts], rsum[:, :],
                    bias_sb[:, bci + tcn * 512:bci + tcn * 512 + 512])

        def v_proj(kt):
            """V for one 128-token tile, directly in [token, dim] layout."""
            tcn, tt = divmod(kt, 4)
            pv = ps_sp.tile([128, 1024], F32, tag="sp", name=f"pv_{kt}")
            for ci in range(CC):
                nc.tensor.matmul(
                    pv[:, 0:DL],
                    xt_sb[tcn][:, ci * 512 + tt * 128:ci * 512 + tt * 128
                               + 128],
                    wv_sb[:, ci * DL:(ci + 1) * DL],
                    start=(ci == 0), stop=(ci == CC - 1))
            nc.vector.tensor_copy(
                vh[:, kt * HL * VW:(kt + 1) * HL * VW]
                .rearrange("p (h j) -> p h j", j=VW)[:, :, 0:HD],
                pv[:, 0:DL].rearrange("p (h j) -> p h j", j=HD))

        def out_proj_tsub(qc, stk, tsub, cast_on_act=False):
            """output projection for one 128-row slice of a done q chunk."""
            row0 = qc * 512 + tsub * 128
            osb = out_pool.tile([128, D], F16, tag="osb", name=f"osb_{row0}")
            op = ps_sp.tile([128, 1024], F32, tag="sp", name=f"op_{row0}")
            for dc in range(2):
                for hp in range(2):
                    nc.tensor.matmul(
                        op[:, dc * 512:(dc + 1) * 512],
                        stk[hp][:, tsub * 128:(tsub + 1) * 128],
                        wo_sb[:, hp * D + dc * 512:hp * D + dc * 512 + 512],
                        start=(hp == 0), stop=(hp == 1))
            if cast_on_act:
                nc.scalar.copy(osb[:, :], op[:, :])
            else:
                nc.vector.tensor_copy(osb[:, :], op[:, :])
            nc.sync.dma_start(out[row0:row0 + 128, :], osb[:, :])

        cn1_of = {}                     # (qc, hp) -> kept cn1 (tail path)

        def norm_steps(cxs, stk_t, tag, keep_cn1=None):
            """normalization for one finished (qc, hp) block, as four
            deferrable steps so the chain never head-of-line-blocks the
            DVE/GpSimd queues at a block boundary."""
            state = {}

            def s1():
                den0 = nrm_pool.tile([1, 1024], F32, tag="den0",
                                     name=f"den0_{tag}")
                nc.sync.dma_start(den0[0:1, :], cxs[HD:HD + 1, :])
                rcp = nrm_pool.tile([1, 1024], F32, tag="rcp",
                                    name=f"rcp_{tag}")
                nc.vector.reciprocal_approx_fast(rcp[0:1, :], den0[0:1, :])
                state["rcp"] = rcp

            def s2():
                bc = nrm_pool.tile([HD, 1024], F32, tag="bc",
                                   name=f"bc_{tag}")
                nc.gpsimd.partition_broadcast(bc[:, :], state["rcp"][0:1, :],
                                              channels=HD)
                state["bc"] = bc

            def s3():
                nc.vector.tensor_mul(stk_t[0:HD, :], cxs[0:HD, 0:512],
                                     state["bc"][:, 0:512])

            def s4():
                if keep_cn1 is not None:
                    # tail path: cn1 is consumed directly as a K=64 chunk
                    # by the final out-projection -- no partition-shift DMA
                    cn1 = stk_pool.tile([HD, 512], BF,
                                        tag=f"cn1t{keep_cn1[1]}", bufs=1,
                                        name=f"cn1_{tag}")
                    cn1_of[keep_cn1] = cn1
                else:
                    cn1 = nrm_pool.tile([HD, 512], BF, tag="cn1",
                                        name=f"cn1_{tag}")
                nc.vector.tensor_mul(cn1[:, :], cxs[0:HD, 512:1024],
                                     state["bc"][:, 512:1024])
                if keep_cn1 is None:
                    nc.sync.dma_start(stk_t[HD:128, :], cn1[:, :])

            return [s1, s2, s3, s4]

        def out_proj_tail(tsub):
            """final q-chunk out-projection: contraction split into four
            K=64 chunks so the h2=1 halves read cn1 at partitions 0:64."""
            row0 = 3 * 512 + tsub * 128
            ts = slice(tsub * 128, (tsub + 1) * 128)
            osb = out_pool.tile([128, D], F16, tag="osb", name=f"osb_{row0}")
            op = ps_sp.tile([128, 1024], F32, tag="sp", name=f"op_{row0}")
            for dc in range(2):
                for hp in range(2):
                    nc.tensor.matmul(
                        op[:, dc * 512:(dc + 1) * 512],
                        stk_of[3][hp][0:HD, ts],
                        wo_sb[0:HD, hp * D + dc * 512:hp * D + dc * 512
                              + 512],
                        start=(hp == 0), stop=False)
                    nc.tensor.matmul(
                        op[:, dc * 512:(dc + 1) * 512],
                        cn1_of[(3, hp)][:, ts],
                        wo_lo[:, hp * D + dc * 512:hp * D + dc * 512 + 512],
                        start=False, stop=(hp == 1))
            nc.scalar.copy(osb[:, :], op[:, :])
            nc.sync.dma_start(out[row0:row0 + 128, :], osb[:, :])

        # warm the PE's HAM clock gate during the input-DMA window with
        # full-array junk matmuls so the first real projections run at
        # 2.4 GHz instead of 1.2 (the monitor ignores near-idle arrays)
        warm = ps_sp.tile([128, 1024], F32, tag="sp", name="warm")
        for _ in range(14):
            nc.tensor.matmul(warm[:, 0:512], ones_bf[:, 0:128],
                             ones_bf[:, 0:512], start=True, stop=True)

        # Only K/Q m-tile 0 of t-chunks 0/1 before attention -- the first
        # head pair's early k-tiles need nothing else, so the exp stream
        # starts ~15us in.  Everything else rides inside the kt loops.
        qk_proj("k", 0, 0, dve_rope=True)
        qk_proj("q", 0, 0, dve_rope=True)
        qk_proj("k", 0, 1, dve_rope=True)
        xt_load(2)
        xt_load(3)
        nc.sync.dma_start(
            wo_sb[:, :].rearrange("p (mt d) -> p mt d", mt=2),
            wo2.rearrange("(mt p) d -> p mt d", p=128))
        nc.sync.dma_start(
            wo_lo[:, :].rearrange("p (mt d) -> p mt d", mt=2),
            wo2.rearrange("(mt p) d -> p mt d", p=128)[HD:128, :, :])

        # remaining projection work for the first attention block, emitted
        # at specific k-tiles (kt4/8/12 score tiles need K(mt0) of t-chunk
        # 1/2/3 a few tiles ahead; hp1 needs all of K(mt1) + Q(mt1))
        K_SCHED = {3: ("k", 0, 2), 5: ("k", 1, 0), 7: ("k", 0, 3),
                   9: ("k", 1, 1), 11: ("k", 1, 2), 12: ("k", 1, 3),
                   13: ("q", 1, 0)}

        # ======== attention ========
        # One flat, software-pipelined stream over all 8 (qc, hp) blocks x
        # 16 k-tiles.  PV lags scores/exp by 2 steps and crosses block
        # boundaries; deferred work (norm steps, out-projections, next-qc
        # Q projections) drains at fixed slots chosen so nothing ever
        # reaches the PE FIFO before its inputs are safely ready.
        blocks = [(qc, hp) for qc in range(4) for hp in range(2)]
        cxs_of = {}                     # bi -> cx psum tile (lazy)
        stk_of = {}                     # qc -> [stk_hp0, stk_hp1]
        todo = []                       # deferred closures
        pend = []                       # (bi, kt, pt) awaiting PV
        # deferred work drains one item per k-tile at kt 1..13; the last
        # two k-tiles of every block stay clean so nothing with a long
        # dependency chain sits in an engine queue across a boundary
        DRAIN_KT = set(range(1, 14))

        def pv(bi, kt, pt):
            qc, hp = blocks[bi]
            if bi not in cxs_of:
                cxs_of[bi] = ps_cx.tile([VW, 1024], F32, tag="cx",
                                        name=f"cx_{qc}_{hp}")
            cx = cxs_of[bi]
            for h2 in range(2):
                hb = (kt * HL + hp * 2 + h2) * VW
                nc.tensor.matmul(
                    cx[:, h2 * 512:(h2 + 1) * 512],
                    vh[:, hb:hb + VW],
                    pt[:, h2 * 512:(h2 + 1) * 512],
                    start=(kt == 0), stop=(kt == NKT - 1))
            if kt == NKT - 1:
                # block finished: one bulk copy frees cx; everything else
                # is deferred (min-kt keeps the out-proj casts away from
                # the next block's critical first k-tiles)
                stk_t = stk_pool.tile([128, 512], BF, tag=f"stk{hp}",
                                      name=f"stk_{qc}_{hp}")
                cxs = nrm_pool.tile([VW, 1024], F32, tag="cxs",
                                    name=f"cxs_{qc}_{hp}")
                nc.vector.tensor_copy(cxs[:, :], cx[:, :])
                todo.extend((1, s) for s in
                            norm_steps(cxs, stk_t, f"{qc}_{hp}",
                                       keep_cn1=((qc, hp) if qc == 3
                                                 else None)))
                stk_of.setdefault(qc, []).append(stk_t)
                if hp == 0 and qc < 3:
                    todo.append((4, lambda qc=qc:
                                 qk_proj("q", 0, qc + 1, dve_rope=True)))
                    todo.append((4, lambda qc=qc:
                                 qk_proj("q", 1, qc + 1, dve_rope=True)))
                if hp == 1:
                    for tsub in range(4):
                        if qc == 3:
                            todo.append((8, lambda tsub=tsub:
                                         out_proj_tail(tsub)))
                        else:
                            todo.append(
                                (7 + 2 * tsub, lambda qc=qc, tsub=tsub:
                                 out_proj_tsub(qc, stk_of[qc], tsub)))

        for bi, (qc, hp) in enumerate(blocks):
            qs = slice(qc * 512, (qc + 1) * 512)
            for kt in range(NKT):
                sp = ps_sp.tile([128, 1024], F32, tag="sp",
                                name=f"sp_{qc}_{hp}_{kt}")
                for h2 in range(2):
                    hs = slice(h2 * HD, (h2 + 1) * HD)
                    nc.tensor.matmul(
                        sp[:, h2 * 512:(h2 + 1) * 512],
                        krot[hp][hs, kt * 128:(kt + 1) * 128],
                        qrot[hp][hs, qs], start=True, stop=True)
                pt = p_pool.tile([128, 1024], BF, tag="pt")
                if bi > 0 and kt in DVE_EXP_KT[hp]:
                    nc.vector.tensor_scalar(
                        pt[:, :].bitcast(I16), sp[:, :],
                        float(SCHRAU_S1), float(SCHRAU_S2),
                        op0=OP.mult, op1=OP.add)
                else:
                    nc.scalar.activation(
                        pt[:, :], sp[:, :], AF.Exp,
                        scale=1.0 / np.sqrt(HD).item())
                if bi == 0:
                    # projection work rides AFTER scores+exp so the exp
                    # stream is never delayed; V(kt-1) still beats its PV
                    # (which lags two steps)
                    if kt in K_SCHED:
                        qk_proj(*K_SCHED[kt], dve_rope=(kt >= 9))
                    if kt >= 1:
                        v_proj(kt - 1)
                    if kt == NKT - 1:
                        v_proj(kt)
                pend.append((bi, kt, pt))
                if len(pend) > 2:
                    pv(*pend.pop(0))
                if bi > 0 and kt in DRAIN_KT:
                    for idx, (min_kt, fn) in enumerate(todo):
                        if kt >= min_kt:
                            todo.pop(idx)
                            fn()
                            break
        while pend:
            pv(*pend.pop(0))
        while todo:
            todo.pop(0)[1]()


def _rope_tables():
    """cos/sin tables in the interleaved-pair partition layout."""
    pos = np.arange(T, dtype=np.float32)[:, None]                 # [T, 1]
    freq_seq = np.arange(HD // 2, dtype=np.float32)
    inv_freq = (1.0 / (10000.0 ** (freq_seq / np.float32(HD // 2)))).astype(
        np.float32)
    ang = pos * inv_freq[None, :]                                 # [T, 32]
    sin = np.sin(ang).astype(np.float32)                          # [T, 32]
    cos = np.cos(ang).astype(np.float32)
    cosb = np.empty((128, T), dtype=np.float32)
    sinb = np.empty((128, T), dtype=np.float32)
    for p in range(128):
        r = p % HD
        j = r // 2
        second = r % 2
        cosb[p] = cos[:, j]
        sinb[p] = sin[:, j] if second else -sin[:, j]
    return cosb, sinb


def _perm():
    """interleaved-pair permutation of each head's 64 dims:
    new[h*64 + 2j] = old[h*64 + j]; new[h*64 + 2j + 1] = old[h*64 + 32 + j]"""
    p = np.arange(DL)
    return (p // HD) * HD + (p % HD) // 2 + (p % 2) * (HD // 2)


def _bf16(a):
    import ml_dtypes
    return np.ascontiguousarray(np.asarray(a, np.float32)).astype(
        ml_dtypes.bfloat16)


def kernel(**inputs):
    global LAST_RESULT
    x = np.asarray(inputs["x"], dtype=np.float32)
    Wq = np.asarray(inputs["Wq"], dtype=np.float32)
    Wk = np.asarray(inputs["Wk"], dtype=np.float32)
    Wv = np.asarray(inputs["Wv"], dtype=np.float32)
    Wo = np.asarray(inputs["Wo"], dtype=np.float32)
    bq = np.asarray(inputs["bq"], dtype=np.float32)
    bk = np.asarray(inputs["bk"], dtype=np.float32)
    bv = np.asarray(inputs["bv"], dtype=np.float32)
    bo = np.asarray(inputs["bo"], dtype=np.float32)

    with_bias = bool(np.any(bq) or np.any(bk))
    key = ("nc", with_bias)
    if key not in _CACHE:
        _CACHE[key] = _build_nc(with_qk_bias=with_bias)
    nc = _CACHE[key]

    xT = [_bf16(x[b].T) for b in range(B)]                        # [D, T]
    cosb, sinb = _rope_tables()
    perm = _perm()
    swap = np.arange(128) ^ 1

    in_maps = []
    for c in range(N_CORES):
        b, hg = divmod(c, 4)
        cs = slice(hg * DL, (hg + 1) * DL)
        im = {
            "xt": xT[b],
            "wq": _bf16(Wq[:, cs][:, perm]),
            "wk": _bf16(Wk[:, cs][:, perm]),
            "wv": _bf16(Wv[:, cs]),
            "wo2": _bf16(Wo[cs, :]),
            "cosb": _bf16(cosb), "sinb": _bf16(sinb),
        }
        if with_bias:
            # additive rope bias tables: b*cos + swap(b)*sin, [mt, q/k]
            bq_c = bq[cs][perm]
            bk_c = bk[cs][perm]
            tabs = []
            for mt in range(2):
                ms = slice(mt * 128, (mt + 1) * 128)
                for b_c in (bq_c, bk_c):
                    tabs.append(b_c[ms][:, None] * cosb
                                + b_c[ms][swap][:, None] * sinb)
            im["bias_t"] = np.concatenate(tabs, axis=1).astype(np.float32)
        in_maps.append(im)

    trace = bool(int(os.environ.get("BASS_KERNEL_TRACE", "0")))
    res = run_bass_kernel_spmd(nc, in_maps, core_ids=list(range(N_CORES)),
                               trace=trace)
    LAST_RESULT = res

    # bv folds into the output bias exactly: ctx includes +bv per head,
    # and sum_h bv_h @ Wo_h = bv @ Wo.
    bo_eff = bo.astype(np.float64) + bv.astype(np.float64) @ Wo.astype(
        np.float64)
    out = np.empty((B, T, D), dtype=np.float32)
    for b in range(B):
        acc = np.zeros((T, D), dtype=np.float64)
        for c in range(4 * b, 4 * b + 4):
            acc += res.results[c]["out"].astype(np.float64)
        out[b] = (acc + bo_eff).astype(np.float32)
    return out


# revision 87
# speedup vs baseline: 1.0183x; 1.0183x over previous
"""Trainium2 Bass kernel for multi-head self-attention with RoPE.

Problem shapes (hardcoded): x [2, 2048, 1024], 16 heads x 64 dim, fp32.
Sharding: 2-D tensor parallel -- core c owns batch c//4 and the 4 heads
[4*(c%4), 4*(c%4)+4).  Each core computes q/k/v projections for its head
quad, RoPE, dense attention over the full 2048x2048 score matrix, and its
partial output projection in bf16; the host sums the 4 fp16 partials per
batch and adds bo (with bv folded in exactly via bo' = bo + bv @ Wo).

Device-side layout notes:
 - Matmul contractions need the contracted dim on SBUF partitions, so the
   host passes x pre-transposed per batch (xt [D, T], bf16).
 - Q^T/K^T are produced in [head_dim, T] bf16 layout; RoPE uses an
   interleaved pair layout (W columns permuted on host) so the rotation
   partner of partition p is p^1 (stream_shuffle within-32 permutation).
   The shuffle + cos-mul + final add run on DVE, the sin mul on GpSimd.
   Nonzero q/k biases are folded into a host-precomputed additive table
   (b*cos + swap(b)*sin); the kernel is built without the extra add when
   the biases are all zero (the build is keyed on that).
 - V is computed directly in [token, dim] layout (x^T chunks stationary)
   so no PE transposes are needed; a strided DVE copy drops the four
   heads' 64-wide slices into a [128, 16*260] arena with interleaved
   ones columns for the softmax denominators.
 - Scores are computed transposed (S^T = K Q^T, [k, q]) with the two heads
   of a pair in different PE row groups; one [128,1024] exp on the scalar
   engine covers both.  PV accumulates ctx^T [65, q] (ones row = denom).
 - Normalization: reciprocal on DVE, partition move via SBUF-SBUF DMA,
   broadcast on GpSimd, multiply on DVE; head 1 of each pair reaches
   partitions 64:128 of the stacked ctx via an SBUF-SBUF DMA.
"""

import os
import numpy as np

import concourse.bass as bass
import concourse.tile as tile
from concourse import mybir
from concourse.bass_utils import run_bass_kernel_spmd

N_CORES = 8
B, T, D = 2, 2048, 1024
H, HD = 16, 64            # total heads, head dim
HL = 4                    # heads per core
DL = HL * HD              # local head dims (256)
CC = D // 128             # contraction chunks (8)
NT = T // 512             # 512-wide t-chunks (4)
NKT = T // 128            # 128-row k-tiles (16)
VW = HD + 1               # per-head vh block width (65)
F32 = mybir.dt.float32
F16 = mybir.dt.float16
BF = mybir.dt.bfloat16
FR = mybir.dt.float32r
I16 = mybir.dt.int16

# within-32 adjacent-pair swap for RoPE (partition p <-> p^1)
SWAP_MASK = [i ^ 1 for i in range(32)]

# k-tiles whose exp runs on DVE via the bf16 Schraudolph bit trick
# (bits = score * 128*log2(e)/8 + (127*128 - C)); C is calibrated so the
# approximation is mean-zero against the exact exp, which makes the
# constant part cancel in the softmax normalization.
# per-hp: chosen to dodge each block's DVE bursts (norm steps at kt 1-4,
# deferred-Q rope at ~4-8 on hp1, out-proj casts at 8-11 on hp0); block 0
# is excluded (its DVE does the V copies)
DVE_EXP_KT = {0: (5, 13), 1: (2, 12)}
SCHRAU_S1 = 128.0 / np.log(2.0) / 8.0
SCHRAU_S2 = 16256.0 - 7.363

_CACHE = {}
LAST_RESULT = None


def _build_nc(with_qk_bias=False, dbg_names=()):
    from concourse import bacc
    nc = bacc.Bacc("TRN2", target_bir_lowering=False, debug=False,
                   num_devices=N_CORES)
    xt = nc.dram_tensor("xt", [D, T], BF, kind="ExternalInput").ap()
    wq = nc.dram_tensor("wq", [D, DL], BF, kind="ExternalInput").ap()
    wk = nc.dram_tensor("wk", [D, DL], BF, kind="ExternalInput").ap()
    wv = nc.dram_tensor("wv", [D, DL], BF, kind="ExternalInput").ap()
    wo2 = nc.dram_tensor("wo2", [DL, D], BF, kind="ExternalInput").ap()
    cosb = nc.dram_tensor("cosb", [128, T], BF, kind="ExternalInput").ap()
    sinb = nc.dram_tensor("sinb", [128, T], BF, kind="ExternalInput").ap()
    # additive rope bias tables per m-tile (b*cos + swap(b)*sin), only
    # read when with_qk_bias: [mt, q/k] -> [128, T]
    bias_t = None
    if with_qk_bias:
        bias_t = nc.dram_tensor("bias_t", [128, 4 * T], F32,
                                kind="ExternalInput").ap()
    out = nc.dram_tensor("out", [T, D], F16, kind="ExternalOutput").ap()

    dbg = {}
    dbg_shapes = {
        "dbg_qrot": [128, T], "dbg_krot": [128, T], "dbg_vh": [128, NKT * 260],
        "dbg_pt": [128, 1024], "dbg_cx": [VW, 512],
    }
    for n in dbg_names:
        dbg[n] = nc.dram_tensor(n, dbg_shapes[n], F32,
                                kind="ExternalOutput").ap()

    with tile.TileContext(nc) as tc:
        _body(tc, xt, wq, wk, wv, wo2, cosb, sinb, bias_t, out, dbg)

    nc.compile()
    return nc


def _body(tc, xt, wq, wk, wv, wo2, cosb, sinb, bias_t, out, dbg={}):
    nc = tc.nc
    AF = mybir.ActivationFunctionType
    OP = mybir.AluOpType
    from contextlib import ExitStack
    with ExitStack() as ctx:
        consts = ctx.enter_context(tc.tile_pool(name="consts", bufs=1))
        persist = ctx.enter_context(tc.tile_pool(name="persist", bufs=1))
        sm_pool = ctx.enter_context(tc.tile_pool(name="sm", bufs=3))
        p_pool = ctx.enter_context(tc.tile_pool(name="pp", bufs=4))
        nrm_pool = ctx.enter_context(tc.tile_pool(name="nrm", bufs=3))
        stk_pool = ctx.enter_context(tc.tile_pool(name="stk", bufs=2))
        out_pool = ctx.enter_context(tc.tile_pool(name="outp", bufs=3))
        # PSUM: ps_sp tag "sp" 3x2 banks + ps_cx 1x2 banks = 8 banks.
        # pp/pv/op all use "sp"-tagged [128,1024] tiles.  cx is single-
        # buffered but freed by one bulk copy right after its last PV.
        ps_sp = ctx.enter_context(tc.tile_pool(name="pse", bufs=3,
                                               space="PSUM"))
        ps_cx = ctx.enter_context(tc.tile_pool(name="psc", bufs=1,
                                               space="PSUM"))

        # ---- constants (DMA order matters: the sync queue is FIFO, and
        # the K projection + first attention block gate the whole span) ----
        wq_sb = consts.tile([128, CC * DL], BF)
        wk_sb = consts.tile([128, CC * DL], BF)
        wv_sb = consts.tile([128, CC * DL], BF)
        wo_sb = consts.tile([128, 2 * D], BF)
        cos_sb = consts.tile([128, T], BF)
        sin_sb = consts.tile([128, T], BF)
        bias_sb = (consts.tile([128, 4 * T], F32)
                   if bias_t is not None else None)

        ones_bf = consts.tile([128, 512], BF)
        nc.gpsimd.memset(ones_bf[:, :], 1.0)

        # persistent per-batch tensors
        qrot = [persist.tile([128, T], BF, tag=f"qrot{mt}",
                             name=f"qrot{mt}") for mt in range(2)]
        krot = [persist.tile([128, T], BF, tag=f"krot{mt}",
                             name=f"krot{mt}") for mt in range(2)]
        vh = persist.tile([128, NKT * HL * VW], BF, tag="vh")
        # ones columns (col 64 of each 65-wide head block)
        nc.gpsimd.memset(
            vh[:, :].rearrange("p (kt h j) -> p kt h j", h=HL, j=VW)
            [:, :, :, HD:HD + 1], 1.0)

        # x^T stays resident in SBUF (16 KiB/partition in bf16), loaded in
        # four t-chunk slices so the K projection can start after the first
        xt_sb = [persist.tile([128, CC * 512], BF, tag=f"xt{tcn}",
                              name=f"xt{tcn}") for tcn in range(NT)]
        xt_src = xt.rearrange("(cc p) t -> p cc t", p=128)

        def xt_load(tcn, split=False):
            # split=True issues odd chunks on the gpsimd DMA queue so the
            # per-instruction issue cost (~0.6us each) parallelizes
            for ci in range(CC):
                eng = nc.gpsimd if (split and ci % 2) else nc.sync
                eng.dma_start(
                    xt_sb[tcn][:, ci * 512:(ci + 1) * 512],
                    xt_src[:, ci, tcn * 512:(tcn + 1) * 512])

        def w_load(w_sb, w, mt, eng=None):
            # one 128-wide m-tile of a projection weight (all CC chunks)
            (eng or nc.sync).dma_start(
                w_sb[:, :].rearrange("p (cc m) -> p cc m", cc=CC)
                [:, :, mt * 128:(mt + 1) * 128],
                w.rearrange("(cc p) m -> p cc m", p=128)
                [:, :, mt * 128:(mt + 1) * 128])

        w_load(wk_sb, wk, 0)
        w_load(wq_sb, wq, 0, eng=nc.gpsimd)
        xt_load(0, split=True)
        nc.sync.dma_start(cos_sb[:, 0:1024], cosb[:, 0:1024])
        nc.sync.dma_start(sin_sb[:, 0:1024], sinb[:, 0:1024])
        nc.sync.dma_start(
            wv_sb[:, :].rearrange("p (cc m) -> p cc m", cc=CC),
            wv.rearrange("(cc p) m -> p cc m", p=128))
        nc.sync.dma_start(cos_sb[:, 1024:T], cosb[:, 1024:T])
        nc.sync.dma_start(sin_sb[:, 1024:T], sinb[:, 1024:T])
        if bias_sb is not None:
            nc.sync.dma_start(bias_sb[:, :], bias_t)
        xt_load(1)
        w_load(wk_sb, wk, 1)
        w_load(wq_sb, wq, 1)

        def qk_proj(name, mt, tcn, dve_rope=False):
            """project one 128-dim m-tile for one 512-col t-chunk + RoPE."""
            w_sb = wq_sb if name == "q" else wk_sb
            dsts = qrot if name == "q" else krot
            ts = slice(tcn * 512, (tcn + 1) * 512)
            pp = ps_sp.tile([128, 1024], F32, tag="sp",
                            name=f"pp_{name}{mt}_{tcn}")
            for ci in range(CC):
                nc.tensor.matmul(
                    pp[:, 0:512],
                    w_sb[:, ci * DL + mt * 128:ci * DL + mt * 128 + 128],
                    xt_sb[tcn][:, ci * 512:(ci + 1) * 512],
                    start=(ci == 0), stop=(ci == CC - 1))
            # rot = pp * cos + swap(pp) * sin [+ bias table]
            # DVE: shuffle + cos-mul (PSUM reads) + add; GpSimd: sin-mul
            shuf = sm_pool.tile([128, 512], F32, tag="shuf")
            nc.vector.stream_shuffle(shuf[:, :], pp[:, 0:512], SWAP_MASK)
            ca = sm_pool.tile([128, 512], BF, tag="ca")
            nc.vector.tensor_mul(ca[:, :], pp[:, 0:512], cos_sb[:, ts])
            sa = sm_pool.tile([128, 512], BF, tag="sa")
            sa_eng = nc.vector if dve_rope else nc.gpsimd
            sa_eng.tensor_mul(sa[:, :], shuf[:, :], sin_sb[:, ts])
            if bias_sb is None:
                nc.vector.tensor_add(dsts[mt][:, ts], ca[:, :], sa[:, :])
            else:
                bci = (mt * 2 + (0 if name == "q" else 1)) * T
                rsum = sm_pool.tile([128, 512], BF, tag="rsum")
                nc.vector.tensor_add(rsum[:, :], ca[:, :], sa[:, :])
                nc.vector.tensor_add(
                    dsts[mt][:, ts], rsum[:, :],
                    bias_sb[:, bci + tcn * 512:bci + tcn * 512 + 512])

        def v_proj(kt):
            """V for one 128-token tile, directly in [token, dim] layout."""
            tcn, tt = divmod(kt, 4)
            pv = ps_sp.tile([128, 1024], F32, tag="sp", name=f"pv_{kt}")
            for ci in range(CC):
                nc.tensor.matmul(
                    pv[:, 0:DL],
                    xt_sb[tcn][:, ci * 512 + tt * 128:ci * 512 + tt * 128
                               + 128],
                    wv_sb[:, ci * DL:(ci + 1) * DL],
                    start=(ci == 0), stop=(ci == CC - 1))
            nc.vector.tensor_copy(
                vh[:, kt * HL * VW:(kt + 1) * HL * VW]
                .rearrange("p (h j) -> p h j", j=VW)[:, :, 0:HD],
                pv[:, 0:DL].rearrange("p (h j) -> p h j", j=HD))

        def out_proj_tsub(qc, stk, tsub, cast_on_act=False):
            """output projection for one 128-row slice of a done q chunk."""
            row0 = qc * 512 + tsub * 128
            osb = out_pool.tile([128, D], F16, tag="osb", name=f"osb_{row0}")
            op = ps_sp.tile([128, 1024], F32, tag="sp", name=f"op_{row0}")
            for dc in range(2):
                for hp in range(2):
                    nc.tensor.matmul(
                        op[:, dc * 512:(dc + 1) * 512],
                        stk[hp][:, tsub * 128:(tsub + 1) * 128],
                        wo_sb[:, hp * D + dc * 512:hp * D + dc * 512 + 512],
                        start=(hp == 0), stop=(hp == 1))
            if cast_on_act:
                nc.scalar.copy(osb[:, :], op[:, :])
            else:
                nc.vector.tensor_copy(osb[:, :], op[:, :])
            nc.sync.dma_start(out[row0:row0 + 128, :], osb[:, :])

        def norm_steps(cxs, stk_t, tag):
            """normalization for one finished (qc, hp) block, as four
            deferrable steps so the chain never head-of-line-blocks the
            DVE/GpSimd queues at a block boundary."""
            state = {}

            def s1():
                den0 = nrm_pool.tile([1, 1024], F32, tag="den0",
                                     name=f"den0_{tag}")
                nc.sync.dma_start(den0[0:1, :], cxs[HD:HD + 1, :])
                rcp = nrm_pool.tile([1, 1024], F32, tag="rcp",
                                    name=f"rcp_{tag}")
                nc.vector.reciprocal_approx_fast(rcp[0:1, :], den0[0:1, :])
                state["rcp"] = rcp

            def s2():
                bc = nrm_pool.tile([HD, 1024], F32, tag="bc",
                                   name=f"bc_{tag}")
                nc.gpsimd.partition_broadcast(bc[:, :], state["rcp"][0:1, :],
                                              channels=HD)
                state["bc"] = bc

            def s3():
                nc.vector.tensor_mul(stk_t[0:HD, :], cxs[0:HD, 0:512],
                                     state["bc"][:, 0:512])

            def s4():
                cn1 = nrm_pool.tile([HD, 512], BF, tag="cn1",
                                    name=f"cn1_{tag}")
                nc.vector.tensor_mul(cn1[:, :], cxs[0:HD, 512:1024],
                                     state["bc"][:, 512:1024])
                nc.sync.dma_start(stk_t[HD:128, :], cn1[:, :])

            return [s1, s2, s3, s4]

        # warm the PE's HAM clock gate during the input-DMA window with
        # full-array junk matmuls so the first real projections run at
        # 2.4 GHz instead of 1.2 (the monitor ignores near-idle arrays)
        warm = ps_sp.tile([128, 1024], F32, tag="sp", name="warm")
        for _ in range(14):
            nc.tensor.matmul(warm[:, 0:512], ones_bf[:, 0:128],
                             ones_bf[:, 0:512], start=True, stop=True)

        # Only K/Q m-tile 0 of t-chunks 0/1 before attention -- the first
        # head pair's early k-tiles need nothing else, so the exp stream
        # starts ~15us in.  Everything else rides inside the kt loops.
        qk_proj("k", 0, 0, dve_rope=True)
        qk_proj("q", 0, 0, dve_rope=True)
        qk_proj("k", 0, 1, dve_rope=True)
        xt_load(2)
        xt_load(3)
        nc.sync.dma_start(
            wo_sb[:, :].rearrange("p (mt d) -> p mt d", mt=2),
            wo2.rearrange("(mt p) d -> p mt d", p=128))

        # remaining projection work for the first attention block, emitted
        # at specific k-tiles (kt4/8/12 score tiles need K(mt0) of t-chunk
        # 1/2/3 a few tiles ahead; hp1 needs all of K(mt1) + Q(mt1))
        K_SCHED = {3: ("k", 0, 2), 5: ("k", 1, 0), 7: ("k", 0, 3),
                   9: ("k", 1, 1), 11: ("k", 1, 2), 12: ("k", 1, 3),
                   13: ("q", 1, 0)}

        # ======== attention ========
        # One flat, software-pipelined stream over all 8 (qc, hp) blocks x
        # 16 k-tiles.  PV lags scores/exp by 2 steps and crosses block
        # boundaries; deferred work (norm steps, out-projections, next-qc
        # Q projections) drains at fixed slots chosen so nothing ever
        # reaches the PE FIFO before its inputs are safely ready.
        blocks = [(qc, hp) for qc in range(4) for hp in range(2)]
        cxs_of = {}                     # bi -> cx psum tile (lazy)
        stk_of = {}                     # qc -> [stk_hp0, stk_hp1]
        todo = []                       # deferred closures
        pend = []                       # (bi, kt, pt) awaiting PV
        # deferred work drains one item per k-tile at kt 1..13; the last
        # two k-tiles of every block stay clean so nothing with a long
        # dependency chain sits in an engine queue across a boundary
        DRAIN_KT = set(range(1, 14))

        def pv(bi, kt, pt):
            qc, hp = blocks[bi]
            if bi not in cxs_of:
                cxs_of[bi] = ps_cx.tile([VW, 1024], F32, tag="cx",
                                        name=f"cx_{qc}_{hp}")
            cx = cxs_of[bi]
            for h2 in range(2):
                hb = (kt * HL + hp * 2 + h2) * VW
                nc.tensor.matmul(
                    cx[:, h2 * 512:(h2 + 1) * 512],
                    vh[:, hb:hb + VW],
                    pt[:, h2 * 512:(h2 + 1) * 512],
                    start=(kt == 0), stop=(kt == NKT - 1))
            if kt == NKT - 1:
                # block finished: one bulk copy frees cx; everything else
                # is deferred (min-kt keeps the out-proj casts away from
                # the next block's critical first k-tiles)
                stk_t = stk_pool.tile([128, 512], BF, tag=f"stk{hp}",
                                      name=f"stk_{qc}_{hp}")
                cxs = nrm_pool.tile([VW, 1024], F32, tag="cxs",
                                    name=f"cxs_{qc}_{hp}")
                nc.vector.tensor_copy(cxs[:, :], cx[:, :])
                todo.extend((1, s) for s in
                            norm_steps(cxs, stk_t, f"{qc}_{hp}"))
                stk_of.setdefault(qc, []).append(stk_t)
                if hp == 0 and qc < 3:
                    todo.append((4, lambda qc=qc:
                                 qk_proj("q", 0, qc + 1, dve_rope=True)))
                    todo.append((4, lambda qc=qc:
                                 qk_proj("q", 1, qc + 1, dve_rope=True)))
                if hp == 1:
                    for tsub in range(4):
                        todo.append(
                            (7 + 2 * tsub, lambda qc=qc, tsub=tsub:
                             out_proj_tsub(qc, stk_of[qc], tsub,
                                           cast_on_act=(qc == 3))))

        for bi, (qc, hp) in enumerate(blocks):
            qs = slice(qc * 512, (qc + 1) * 512)
            for kt in range(NKT):
                sp = ps_sp.tile([128, 1024], F32, tag="sp",
                                name=f"sp_{qc}_{hp}_{kt}")
                for h2 in range(2):
                    hs = slice(h2 * HD, (h2 + 1) * HD)
                    nc.tensor.matmul(
                        sp[:, h2 * 512:(h2 + 1) * 512],
                        krot[hp][hs, kt * 128:(kt + 1) * 128],
                        qrot[hp][hs, qs], start=True, stop=True)
                pt = p_pool.tile([128, 1024], BF, tag="pt")
                if bi > 0 and kt in DVE_EXP_KT[hp]:
                    nc.vector.tensor_scalar(
                        pt[:, :].bitcast(I16), sp[:, :],
                        float(SCHRAU_S1), float(SCHRAU_S2),
                        op0=OP.mult, op1=OP.add)
                else:
                    nc.scalar.activation(
                        pt[:, :], sp[:, :], AF.Exp,
                        scale=1.0 / np.sqrt(HD).item())
                if bi == 0:
                    # projection work rides AFTER scores+exp so the exp
                    # stream is never delayed; V(kt-1) still beats its PV
                    # (which lags two steps)
                    if kt in K_SCHED:
                        qk_proj(*K_SCHED[kt], dve_rope=(kt >= 9))
                    if kt >= 1:
                        v_proj(kt - 1)
                    if kt == NKT - 1:
                        v_proj(kt)
                pend.append((bi, kt, pt))
                if len(pend) > 2:
                    pv(*pend.pop(0))
                if bi > 0 and kt in DRAIN_KT:
                    for idx, (min_kt, fn) in enumerate(todo):
                        if kt >= min_kt:
                            todo.pop(idx)
                            fn()
                            break
        while pend:
            pv(*pend.pop(0))
        while todo:
            todo.pop(0)[1]()


def _rope_tables():
    """cos/sin tables in the interleaved-pair partition layout."""
    pos = np.arange(T, dtype=np.float32)[:, None]                 # [T, 1]
    freq_seq = np.arange(HD // 2, dtype=np.float32)
    inv_freq = (1.0 / (10000.0 ** (freq_seq / np.float32(HD // 2)))).astype(
        np.float32)
    ang = pos * inv_freq[None, :]                                 # [T, 32]
    sin = np.sin(ang).astype(np.float32)                          # [T, 32]
    cos = np.cos(ang).astype(np.float32)
    cosb = np.empty((128, T), dtype=np.float32)
    sinb = np.empty((128, T), dtype=np.float32)
    for p in range(128):
        r = p % HD
        j = r // 2
        second = r % 2
        cosb[p] = cos[:, j]
        sinb[p] = sin[:, j] if second else -sin[:, j]
    return cosb, sinb


def _perm():
    """interleaved-pair permutation of each head's 64 dims:
    new[h*64 + 2j] = old[h*64 + j]; new[h*64 + 2j + 1] = old[h*64 + 32 + j]"""
    p = np.arange(DL)
    return (p // HD) * HD + (p % HD) // 2 + (p % 2) * (HD // 2)


def _bf16(a):
    import ml_dtypes
    return np.ascontiguousarray(np.asarray(a, np.float32)).astype(
        ml_dtypes.bfloat16)


def kernel(**inputs):
    global LAST_RESULT
    x = np.asarray(inputs["x"], dtype=np.float32)
    Wq = np.asarray(inputs["Wq"], dtype=np.float32)
    Wk = np.asarray(inputs["Wk"], dtype=np.float32)
    Wv = np.asarray(inputs["Wv"], dtype=np.float32)
    Wo = np.asarray(inputs["Wo"], dtype=np.float32)
    bq = np.asarray(inputs["bq"], dtype=np.float32)
    bk = np.asarray(inputs["bk"], dtype=np.float32)
    bv = np.asarray(inputs["bv"], dtype=np.float32)
    bo = np.asarray(inputs["bo"], dtype=np.float32)

    with_bias = bool(np.any(bq) or np.any(bk))
    key = ("nc", with_bias)
    if key not in _CACHE:
        _CACHE[key] = _build_nc(with_qk_bias=with_bias)
    nc = _CACHE[key]

    xT = [_bf16(x[b].T) for b in range(B)]                        # [D, T]
    cosb, sinb = _rope_tables()
    perm = _perm()
    swap = np.arange(128) ^ 1

    in_maps = []
    for c in range(N_CORES):
        b, hg = divmod(c, 4)
        cs = slice(hg * DL, (hg + 1) * DL)
        im = {
            "xt": xT[b],
            "wq": _bf16(Wq[:, cs][:, perm]),
            "wk": _bf16(Wk[:, cs][:, perm]),
            "wv": _bf16(Wv[:, cs]),
            "wo2": _bf16(Wo[cs, :]),
            "cosb": _bf16(cosb), "sinb": _bf16(sinb),
        }
        if with_bias:
            # additive rope bias tables: b*cos + swap(b)*sin, [mt, q/k]
            bq_c = bq[cs][perm]
            bk_c = bk[cs][perm]
            tabs = []
            for mt in range(2):
                ms = slice(mt * 128, (mt + 1) * 128)
                for b_c in (bq_c, bk_c):
                    tabs.append(b_c[ms][:, None] * cosb
                                + b_c[ms][swap][:, None] * sinb)
            im["bias_t"] = np.concatenate(tabs, axis=1).astype(np.float32)
        in_maps.append(im)

    trace = bool(int(os.environ.get("BASS_KERNEL_TRACE", "0")))
    res = run_bass_kernel_spmd(nc, in_maps, core_ids=list(range(N_CORES)),
                               trace=trace)
    LAST_RESULT = res

    # bv folds into the output bias exactly: ctx includes +bv per head,
    # and sum_h bv_h @ Wo_h = bv @ Wo.
    bo_eff = bo.astype(np.float64) + bv.astype(np.float64) @ Wo.astype(
        np.float64)
    out = np.empty((B, T, D), dtype=np.float32)
    for b in range(B):
        acc = np.zeros((T, D), dtype=np.float64)
        for c in range(4 * b, 4 * b + 4):
            acc += res.results[c]["out"].astype(np.float64)
        out[b] = (acc + bo_eff).astype(np.float32)
    return out
